# revision 2
# baseline (speedup 1.0000x reference)
"""LowPassMSELoss Trainium2 kernel (v10: v2 pipeline, no pad refetch, no scr dance).

Math: loss = mean((lfilter(b,a,o) - lfilter(b,a,t))^2)
    = mean(conv(o-t, h)^2), h = impulse response truncated to K=128 taps.

v2's dense-DVE pipeline (single sync-ring stream, row-major compute) kept
verbatim -- engine clocks are activity-throttled, so the dense DVE/PE queues
run at the faster pstate -- with two deltas:
  - chunk order per row is 3,0,1,2 and the pad cols come from chunk 3's own
    transpose (block q=3 = blocks 16n+15): the two pad-refetch DMAs, their
    sub and their transpose are gone (~0.26 MB less stream traffic)
  - all 8 y tiles live in 6 psum banks with in-place ACT squares; the two
    recycled banks' [DVE data + ACT WAR] conv waits are legalized by
    splitting into same-engine Drain carriers (no scr copies: ~2.4us less
    DVE work, which also shortens the DVE-paced drain)
"""

import os
import ml_dtypes
import numpy as np

B, T = 16, 262144
NCORES = 8
ROWS_PER_CORE = B // NCORES          # 2
F = 2048                             # free dim of natural layout (T / 128)
K = 128                              # FIR taps
NJ = F // 512                        # 4 conv tiles (= chunks) per row
XBW = 128 + F                        # xb width (128 pad cols + data)

last_exec_time_ns = None
_CACHE = {}


def _impulse_response(b, a, n):
    """First n samples of the IIR impulse response, float64, DF2T like scipy."""
    b = np.asarray(b, np.float64)
    a = np.asarray(a, np.float64)
    b = b / a[0]
    a = a / a[0]
    order = len(a) - 1
    z = np.zeros(order, np.float64)
    h = np.empty(n, np.float64)
    for i in range(n):
        x = 1.0 if i == 0 else 0.0
        y = b[0] * x + z[0]
        znew = np.empty(order, np.float64)
        znew[: order - 1] = z[1:] + b[1:order] * x - a[1:order] * y
        znew[order - 1] = b[order] * x - a[order] * y
        z = znew
        h[i] = y
    return h


def _toeplitz_lhsts(h):
    """lhsT_A[i,j] = h[j-i] (j>=i), lhsT_B[i,j] = h[128+j-i] (i>j)."""
    i = np.arange(K)[:, None]
    j = np.arange(K)[None, :]
    dj = j - i
    A = np.where(dj >= 0, h[np.clip(dj, 0, K - 1)], 0.0)
    Bm = np.where(dj < 0, h[np.clip(K + dj, 0, K - 1)], 0.0)
    return A, Bm


def _drop_vacuous_self_waits(nc):
    """trn2 codegen allows one sync-wait per instruction; Tile sometimes
    attaches a same-engine self-wait alongside a foreign one.  Engine queues
    issue in order and every same-engine op increments the engine sem, so a
    self-wait whose threshold is already guaranteed by queue position is
    droppable."""
    import copy

    prior_incs = {}
    for f in nc.m.functions:
        for bb in f.blocks:
            new_list = []
            for ins in bb.instructions:
                si = ins.sync_info
                if (
                    si is not None
                    and si.on_wait
                    and len(si.on_wait) > 1
                    and "Drain" in type(ins).__name__
                ):
                    waits = list(si.on_wait)
                    for k, w in enumerate(waits[:-1]):
                        pre = copy.deepcopy(ins)
                        pre.name = f"{ins.name}-w{k}"
                        pre.sync_info = copy.deepcopy(si)
                        pre.sync_info.on_wait = [w]
                        pre.sync_info.on_update = []
                        new_list.append(pre)
                    si.on_wait = [waits[-1]]
                new_list.append(ins)
            bb.instructions = new_list
    for f in nc.m.functions:
        for bb in f.blocks:
            for ins in bb.instructions:
                si = ins.sync_info
                if si is None:
                    continue
                waits = list(si.on_wait or [])
                if len(waits) > 1:
                    kept = []
                    for w in waits:
                        name = getattr(w, "ant_name", "") or ""
                        eng = getattr(getattr(ins, "engine", None), "value", "zz")
                        if (
                            name.startswith(eng)
                            and prior_incs.get(name, 0) >= (w.wait_value or 0)
                        ):
                            continue
                        kept.append(w)
                    si.on_wait = kept
                for u in si.on_update or []:
                    name = getattr(u, "ant_name", "") or ""
                    if name:
                        prior_incs[name] = prior_incs.get(name, 0) + (
                            u.update_value or 1
                        )


def _split_multi_waits(nc, mybir):
    """Split any instruction holding N>1 sync-waits into N-1 preceding
    same-engine Drain carriers (one wait each).  A Drain must not separate
    an Ldweights from its Matmult (walrus pairs them), so carriers hop back
    over trailing same-engine Ldweights."""
    import copy

    n_split = 0
    for f in nc.m.functions:
        for bb in f.blocks:
            new_list = []
            for ins in bb.instructions:
                si = ins.sync_info
                if si is not None and si.on_wait and len(si.on_wait) > 1:
                    pos = len(new_list)
                    while (
                        pos > 0
                        and type(new_list[pos - 1]).__name__ == "InstLdweights"
                        and new_list[pos - 1].engine == ins.engine
                    ):
                        pos -= 1
                    waits = list(si.on_wait)
                    for k, w in enumerate(waits[:-1]):
                        drain = mybir.InstDrain(
                            name=f"{ins.name}-wait{k}",
                            engine=ins.engine,
                        )
                        drain.sync_info = copy.deepcopy(si)
                        drain.sync_info.on_wait = [w]
                        drain.sync_info.on_update = []
                        new_list.insert(pos, drain)
                        pos += 1
                        n_split += 1
                    si.on_wait = [waits[-1]]
                new_list.append(ins)
            bb.instructions = new_list
    return n_split


def _build_bass():
    import concourse.bass as bass
    import concourse.tile as tile
    from concourse import mybir

    dt = mybir.dt
    nc = bass.Bass(trn_type="TRN2")

    ot_h = nc.dram_tensor(
        "ot", [ROWS_PER_CORE, 2, T], dt.float32, kind="ExternalInput"
    )
    # host packs consts partition-major: C_h[p, 128c+f] = (A,B,I)[c][p,f],
    # so the DMA is 128 contiguous 768B descriptors
    C_h = nc.dram_tensor("consts", [K, 3 * K], dt.bfloat16, kind="ExternalInput")
    out_h = nc.dram_tensor(
        "partials", [128, ROWS_PER_CORE * NJ], dt.float32, kind="ExternalOutput"
    )

    # ot4[r, p, s, f] = ot[r, s, 2048p + f]
    ot4 = ot_h[:].rearrange("b s (p f) -> b p s f", p=128)

    CHUNK_ORDER = [3, 0, 1, 2]

    with tile.TileContext(nc) as tc:
        with (
            tc.tile_pool(name="consts", bufs=1) as consts,
            tc.tile_pool(name="io", bufs=2 * NJ * ROWS_PER_CORE) as io_pool,
            tc.tile_pool(name="dpool", bufs=3) as dpool,
            tc.tile_pool(name="xb", bufs=ROWS_PER_CORE) as xbpool,
            tc.tile_pool(name="ptr", bufs=2, space="PSUM") as ptr_pool,
            tc.tile_pool(name="ya", bufs=6, space="PSUM") as ya_pool,
            tc.tile_pool(name="outp", bufs=1) as out_pool,
        ):
            # input chunks first on the queue: data starts flowing ASAP;
            # row-major order matching the row-major compute below, chunk 3
            # first within each row (pad source)
            io_tiles = {}
            first = True
            for r in range(ROWS_PER_CORE):
                for c in CHUNK_ORDER:
                    t_io = io_pool.tile(
                        [128, 2, 512], dt.float32, tag="ot", name="ot"
                    )
                    nc.sync.dma_start(t_io[:], ot4[r][:, :, 512 * c : 512 * (c + 1)])
                    io_tiles[(r, c)] = t_io
                    if first:
                        c_raw = consts.tile(
                            [K, 3, K], dt.bfloat16, tag="Craw", name="Craw"
                        )
                        nc.sync.dma_start(
                            c_raw[:],
                            C_h[:].rearrange("p (c f) -> p c f", c=3),
                        )
                        first = False
            # funnel the const-DMA dep through DVE so PE ops wait on one engine
            c_sb = consts.tile([K, 3, K], dt.bfloat16, tag="C", name="C")
            nc.vector.tensor_copy(c_sb[:], c_raw[:])
            A_sb = c_sb[:, 0, :]
            B_sb = c_sb[:, 1, :]
            I_sb = c_sb[:, 2, :]

            out_sb = out_pool.tile(
                [128, ROWS_PER_CORE * NJ], dt.float32, name="outsb"
            )

            def conv_tile(r, c):
                xb = xbs
                py = ya_pool.tile([128, 512], dt.float32, tag="y", name="y")
                nc.tensor.matmul(
                    py[:],
                    B_sb[:],
                    xb[:, 512 * c : 512 * (c + 1)],
                    start=True,
                    stop=False,
                )
                nc.tensor.matmul(
                    py[:],
                    A_sb[:],
                    xb[:, 128 + 512 * c : 128 + 512 * (c + 1)],
                    start=False,
                    stop=True,
                )
                acc = out_sb[:, NJ * r + c : NJ * r + c + 1]
                nc.scalar.activation(
                    py[:],
                    py[:],
                    mybir.ActivationFunctionType.Square,
                    scale=1.0,
                    accum_out=acc,
                )

            for r in range(ROWS_PER_CORE):
                xbs = xbpool.tile([128, XBW], dt.bfloat16, tag="xb", name="xb")

                for c in CHUNK_ORDER:
                    t_io = io_tiles[(r, c)]
                    d16 = dpool.tile([128, 512], dt.bfloat16, tag="d", name="d")
                    nc.vector.tensor_sub(d16[:], t_io[:, 0, :], t_io[:, 1, :])

                    ptr = ptr_pool.tile(
                        [128, 512], dt.float32, tag="tr", name="tr"
                    )
                    for q in range(4):
                        nc.tensor.matmul(
                            ptr[:, 128 * q : 128 * (q + 1)],
                            d16[:, 128 * q : 128 * (q + 1)],
                            I_sb[:],
                            start=True,
                            stop=True,
                        )
                    dst = xbs[:, 128 + 512 * c : 128 + 512 * (c + 1)]
                    nc.vector.tensor_copy(dst, ptr[:])
                    if c == 3:
                        # pad: col p holds block 16p-1; block q=3 of this
                        # chunk's transpose is blocks 16n+15, shifted one col
                        nc.vector.memset(xbs[:, 0:1], 0.0)
                        nc.vector.tensor_copy(xbs[:, 1:128], ptr[:, 384:511])
                    else:
                        conv_tile(r, c)
                        if c == 2:
                            conv_tile(r, 3)

            # issue from ACT's HWDGE queue: the dep on ACT's accum writes is
            # implicit in program order, keeping this under the 1-wait limit
            nc.scalar.dma_start(out_h[:], out_sb[:])

    _drop_vacuous_self_waits(nc)
    _split_multi_waits(nc, mybir)
    return nc


def kernel(output, target, b, a):
    global last_exec_time_ns
    from concourse.bass_utils import run_bass_kernel_spmd

    output = np.asarray(output, np.float32)
    target = np.asarray(target, np.float32)

    if "nc" not in _CACHE:
        _CACHE["nc"] = _build_bass()
    nc = _CACHE["nc"]

    h = _impulse_response(np.asarray(b, np.float64), np.asarray(a, np.float64), K)
    A_m, B_m = _toeplitz_lhsts(h)
    # partition-major packing: consts[p, 128c+f] = (A,B,I)[c][p,f]
    consts = np.ascontiguousarray(
        np.stack([A_m, B_m, np.eye(K)]).transpose(1, 0, 2).reshape(K, 3 * K)
    ).astype(ml_dtypes.bfloat16)

    ot = np.stack([output, target], axis=1)  # [B, 2, T]
    in_maps = []
    for c in range(NCORES):
        rows = slice(c * ROWS_PER_CORE, (c + 1) * ROWS_PER_CORE)
        in_maps.append(
            {
                "ot": np.ascontiguousarray(ot[rows]),
                "consts": consts,
            }
        )

    res = run_bass_kernel_spmd(
        nc,
        in_maps,
        core_ids=list(range(NCORES)),
        trace=bool(int(os.environ.get("LP_TRACE", "0"))),
    )
    last_exec_time_ns = res.exec_time_ns

    total = np.float64(0.0)
    for r in res.results:
        total += r["partials"].astype(np.float64).sum()
    return np.float32(total / (B * T))


# revision 3
# speedup vs baseline: 1.0192x; 1.0192x over previous
"""LowPassMSELoss Trainium2 kernel (v17: v10 + serialized output).

Math: loss = mean((lfilter(b,a,o) - lfilter(b,a,t))^2)
    = mean(conv(o-t, h)^2), h = impulse response truncated to K=128 taps.

v2's dense-DVE pipeline (single sync-ring stream, row-major compute) kept
verbatim -- engine clocks are activity-throttled, so the dense DVE/PE queues
run at the faster pstate -- with two deltas:
  - chunk order per row is 3,0,1,2 and the pad cols come from chunk 3's own
    transpose (block q=3 = blocks 16n+15): the two pad-refetch DMAs, their
    sub and their transpose are gone (~0.26 MB less stream traffic)
  - all 8 y tiles live in 6 psum banks with in-place ACT squares; the two
    recycled banks' [DVE data + ACT WAR] conv waits are legalized by
    splitting into same-engine Drain carriers (no scr copies: ~2.4us less
    DVE work, which also shortens the DVE-paced drain)
"""

import os
import ml_dtypes
import numpy as np

B, T = 16, 262144
NCORES = 8
ROWS_PER_CORE = B // NCORES          # 2
F = 2048                             # free dim of natural layout (T / 128)
K = 128                              # FIR taps
NJ = F // 512                        # 4 conv tiles (= chunks) per row
XBW = 128 + F                        # xb width (128 pad cols + data)

last_exec_time_ns = None
_CACHE = {}


def _impulse_response(b, a, n):
    """First n samples of the IIR impulse response, float64, DF2T like scipy."""
    b = np.asarray(b, np.float64)
    a = np.asarray(a, np.float64)
    b = b / a[0]
    a = a / a[0]
    order = len(a) - 1
    z = np.zeros(order, np.float64)
    h = np.empty(n, np.float64)
    for i in range(n):
        x = 1.0 if i == 0 else 0.0
        y = b[0] * x + z[0]
        znew = np.empty(order, np.float64)
        znew[: order - 1] = z[1:] + b[1:order] * x - a[1:order] * y
        znew[order - 1] = b[order] * x - a[order] * y
        z = znew
        h[i] = y
    return h


def _toeplitz_lhsts(h):
    """lhsT_A[i,j] = h[j-i] (j>=i), lhsT_B[i,j] = h[128+j-i] (i>j)."""
    i = np.arange(K)[:, None]
    j = np.arange(K)[None, :]
    dj = j - i
    A = np.where(dj >= 0, h[np.clip(dj, 0, K - 1)], 0.0)
    Bm = np.where(dj < 0, h[np.clip(K + dj, 0, K - 1)], 0.0)
    return A, Bm


def _drop_vacuous_self_waits(nc):
    """trn2 codegen allows one sync-wait per instruction; Tile sometimes
    attaches a same-engine self-wait alongside a foreign one.  Engine queues
    issue in order and every same-engine op increments the engine sem, so a
    self-wait whose threshold is already guaranteed by queue position is
    droppable."""
    import copy

    prior_incs = {}
    for f in nc.m.functions:
        for bb in f.blocks:
            new_list = []
            for ins in bb.instructions:
                si = ins.sync_info
                if (
                    si is not None
                    and si.on_wait
                    and len(si.on_wait) > 1
                    and "Drain" in type(ins).__name__
                ):
                    waits = list(si.on_wait)
                    for k, w in enumerate(waits[:-1]):
                        pre = copy.deepcopy(ins)
                        pre.name = f"{ins.name}-w{k}"
                        pre.sync_info = copy.deepcopy(si)
                        pre.sync_info.on_wait = [w]
                        pre.sync_info.on_update = []
                        new_list.append(pre)
                    si.on_wait = [waits[-1]]
                new_list.append(ins)
            bb.instructions = new_list
    for f in nc.m.functions:
        for bb in f.blocks:
            for ins in bb.instructions:
                si = ins.sync_info
                if si is None:
                    continue
                waits = list(si.on_wait or [])
                if len(waits) > 1:
                    kept = []
                    for w in waits:
                        name = getattr(w, "ant_name", "") or ""
                        eng = getattr(getattr(ins, "engine", None), "value", "zz")
                        if (
                            name.startswith(eng)
                            and prior_incs.get(name, 0) >= (w.wait_value or 0)
                        ):
                            continue
                        kept.append(w)
                    si.on_wait = kept
                for u in si.on_update or []:
                    name = getattr(u, "ant_name", "") or ""
                    if name:
                        prior_incs[name] = prior_incs.get(name, 0) + (
                            u.update_value or 1
                        )


def _split_multi_waits(nc, mybir):
    """Split any instruction holding N>1 sync-waits into N-1 preceding
    same-engine Drain carriers (one wait each).  A Drain must not separate
    an Ldweights from its Matmult (walrus pairs them), so carriers hop back
    over trailing same-engine Ldweights."""
    import copy

    n_split = 0
    for f in nc.m.functions:
        for bb in f.blocks:
            new_list = []
            for ins in bb.instructions:
                si = ins.sync_info
                if si is not None and si.on_wait and len(si.on_wait) > 1:
                    pos = len(new_list)
                    while (
                        pos > 0
                        and type(new_list[pos - 1]).__name__ == "InstLdweights"
                        and new_list[pos - 1].engine == ins.engine
                    ):
                        pos -= 1
                    waits = list(si.on_wait)
                    for k, w in enumerate(waits[:-1]):
                        drain = mybir.InstDrain(
                            name=f"{ins.name}-wait{k}",
                            engine=ins.engine,
                        )
                        drain.sync_info = copy.deepcopy(si)
                        drain.sync_info.on_wait = [w]
                        drain.sync_info.on_update = []
                        new_list.insert(pos, drain)
                        pos += 1
                        n_split += 1
                    si.on_wait = [waits[-1]]
                new_list.append(ins)
            bb.instructions = new_list
    return n_split


def _build_bass():
    import concourse.bass as bass
    import concourse.tile as tile
    from concourse import mybir

    dt = mybir.dt
    nc = bass.Bass(trn_type="TRN2")

    ot_h = nc.dram_tensor(
        "ot", [ROWS_PER_CORE, 2, T], dt.float32, kind="ExternalInput"
    )
    # host packs consts partition-major: C_h[p, 128c+f] = (A,B,I)[c][p,f],
    # so the DMA is 128 contiguous 768B descriptors
    C_h = nc.dram_tensor("consts", [K, 3 * K], dt.bfloat16, kind="ExternalInput")
    out_h = nc.dram_tensor(
        "partials", [128, ROWS_PER_CORE * NJ], dt.float32, kind="ExternalOutput"
    )

    # ot4[r, p, s, f] = ot[r, s, 2048p + f]
    ot4 = ot_h[:].rearrange("b s (p f) -> b p s f", p=128)

    CHUNK_ORDER = [3, 0, 1, 2]

    with tile.TileContext(nc) as tc:
        with (
            tc.tile_pool(name="consts", bufs=1) as consts,
            tc.tile_pool(name="io", bufs=2 * NJ * ROWS_PER_CORE) as io_pool,
            tc.tile_pool(name="dpool", bufs=3) as dpool,
            tc.tile_pool(name="xb", bufs=ROWS_PER_CORE) as xbpool,
            tc.tile_pool(name="ptr", bufs=2, space="PSUM") as ptr_pool,
            tc.tile_pool(name="ya", bufs=6, space="PSUM") as ya_pool,
            tc.tile_pool(name="outp", bufs=1) as out_pool,
        ):
            # input chunks first on the queue: data starts flowing ASAP;
            # row-major order matching the row-major compute below, chunk 3
            # first within each row (pad source)
            io_tiles = {}
            first = True
            for r in range(ROWS_PER_CORE):
                for c in CHUNK_ORDER:
                    t_io = io_pool.tile(
                        [128, 2, 512], dt.float32, tag="ot", name="ot"
                    )
                    nc.sync.dma_start(t_io[:], ot4[r][:, :, 512 * c : 512 * (c + 1)])
                    io_tiles[(r, c)] = t_io
                    if first:
                        c_raw = consts.tile(
                            [K, 3, K], dt.bfloat16, tag="Craw", name="Craw"
                        )
                        nc.sync.dma_start(
                            c_raw[:],
                            C_h[:].rearrange("p (c f) -> p c f", c=3),
                        )
                        first = False
            # funnel the const-DMA dep through DVE so PE ops wait on one engine
            c_sb = consts.tile([K, 3, K], dt.bfloat16, tag="C", name="C")
            nc.vector.tensor_copy(c_sb[:], c_raw[:])
            A_sb = c_sb[:, 0, :]
            B_sb = c_sb[:, 1, :]
            I_sb = c_sb[:, 2, :]

            out_sb = out_pool.tile(
                [128, ROWS_PER_CORE * NJ], dt.float32, name="outsb"
            )

            def conv_tile(r, c):
                xb = xbs
                py = ya_pool.tile([128, 512], dt.float32, tag="y", name="y")
                nc.tensor.matmul(
                    py[:],
                    B_sb[:],
                    xb[:, 512 * c : 512 * (c + 1)],
                    start=True,
                    stop=False,
                )
                nc.tensor.matmul(
                    py[:],
                    A_sb[:],
                    xb[:, 128 + 512 * c : 128 + 512 * (c + 1)],
                    start=False,
                    stop=True,
                )
                acc = out_sb[:, NJ * r + c : NJ * r + c + 1]
                nc.scalar.activation(
                    py[:],
                    py[:],
                    mybir.ActivationFunctionType.Square,
                    scale=1.0,
                    accum_out=acc,
                )

            for r in range(ROWS_PER_CORE):
                xbs = xbpool.tile([128, XBW], dt.bfloat16, tag="xb", name="xb")

                for c in CHUNK_ORDER:
                    t_io = io_tiles[(r, c)]
                    d16 = dpool.tile([128, 512], dt.bfloat16, tag="d", name="d")
                    nc.vector.tensor_sub(d16[:], t_io[:, 0, :], t_io[:, 1, :])

                    ptr = ptr_pool.tile(
                        [128, 512], dt.float32, tag="tr", name="tr"
                    )
                    for q in range(4):
                        nc.tensor.matmul(
                            ptr[:, 128 * q : 128 * (q + 1)],
                            d16[:, 128 * q : 128 * (q + 1)],
                            I_sb[:],
                            start=True,
                            stop=True,
                        )
                    dst = xbs[:, 128 + 512 * c : 128 + 512 * (c + 1)]
                    nc.vector.tensor_copy(dst, ptr[:])
                    if c == 3:
                        # pad: col p holds block 16p-1; block q=3 of this
                        # chunk's transpose is blocks 16n+15, shifted one col
                        nc.vector.memset(xbs[:, 0:1], 0.0)
                        nc.vector.tensor_copy(xbs[:, 1:128], ptr[:, 384:511])
                    else:
                        conv_tile(r, c)
                        if c == 2:
                            conv_tile(r, 3)

            # The accumulator lands via a separate READ_ACCUMULATOR ACT
            # instruction after each Square; a dma_start dispatched right
            # after the last square can race it (observed as an intermittent
            # rel-err jump 1.2e-3 -> 1.5e-2).  Serialize through an ACT copy:
            # same-engine in-order execution makes the copy's read safe, and
            # the DMA then reads a tile whose (tiny) writer completes well
            # inside the DMA's first-byte latency.
            out2 = out_pool.tile(
                [128, ROWS_PER_CORE * NJ], dt.float32, name="out2"
            )
            nc.scalar.copy(out2[:], out_sb[:])
            nc.scalar.dma_start(out_h[:], out2[:])

    _drop_vacuous_self_waits(nc)
    _split_multi_waits(nc, mybir)
    return nc


def kernel(output, target, b, a):
    global last_exec_time_ns
    from concourse.bass_utils import run_bass_kernel_spmd

    output = np.asarray(output, np.float32)
    target = np.asarray(target, np.float32)

    if "nc" not in _CACHE:
        _CACHE["nc"] = _build_bass()
    nc = _CACHE["nc"]

    h = _impulse_response(np.asarray(b, np.float64), np.asarray(a, np.float64), K)
    A_m, B_m = _toeplitz_lhsts(h)
    # partition-major packing: consts[p, 128c+f] = (A,B,I)[c][p,f]
    consts = np.ascontiguousarray(
        np.stack([A_m, B_m, np.eye(K)]).transpose(1, 0, 2).reshape(K, 3 * K)
    ).astype(ml_dtypes.bfloat16)

    ot = np.stack([output, target], axis=1)  # [B, 2, T]
    in_maps = []
    for c in range(NCORES):
        rows = slice(c * ROWS_PER_CORE, (c + 1) * ROWS_PER_CORE)
        in_maps.append(
            {
                "ot": np.ascontiguousarray(ot[rows]),
                "consts": consts,
            }
        )

    res = run_bass_kernel_spmd(
        nc,
        in_maps,
        core_ids=list(range(NCORES)),
        trace=bool(int(os.environ.get("LP_TRACE", "0"))),
    )
    last_exec_time_ns = res.exec_time_ns

    total = np.float64(0.0)
    for r in res.results:
        total += r["partials"].astype(np.float64).sum()
    return np.float32(total / (B * T))
